# revision 31
# baseline (speedup 1.0000x reference)
"""Trainium2 Bass kernel for nn_Attention_7275674600158.

Sharding: 8 cores = 2-way data parallel over batch x 4-way tensor parallel
over KV-head groups (4 q-heads + 1 kv-head per core). Each core computes a
partial output [S, D] (contribution of its 4 heads); host sums the 4 partials
per batch element.
"""

import numpy as np

B, S, D = 2, 2048, 1024
H, HKV, HD = 16, 4, 64
EPS = 1e-5
P = 128
NT = S // P   # 16 token tiles
CH = 512      # q chunk
NCH = S // CH  # 4 chunks
ND = D // P   # 8 d blocks
HG = H // HKV  # 4 q heads per group
LOOP_K = 256  # executions per invocation in the bench (loop) variant

_CACHE = {}


def _install_tile_patch():
    """This walrus build encodes only 1 sync-wait per CTRL instruction; split
    the Tile epilogue drain's waits across one pre-drain per busy proc."""
    import concourse.tile as _tm
    from concourse.vector_clock import ScopedClock, VectorClock

    if getattr(_tm.TileContext, "_drain_split_patch", False):
        return

    def _split(self, tick_clock, wait_clock):
        vals = list(tick_clock.global_clock)
        for p, v in enumerate(vals):
            if v > 0:
                vc = VectorClock()
                vc.require_at_least(p, v)
                d = self.nc.sync.drain()
                wait_clock.add_sem_waits(d.ins, ScopedClock({None: vc}))
        self.nc.sync.drain()
        self.nc.all_engine_barrier()
        popped = self.nc._tile_sem_poison_stack.pop()
        assert popped is self._sem_poison
        self.nc.clear_and_free_semaphores(list(self.sems.allocated().values()))
        self.nc.all_engine_barrier()

    _tm.TileContext._drain_and_barrier = _split
    _tm.TileContext._drain_split_patch = True


def _split_multi_waits(nc):
    """walrus here encodes only one sync-wait per instruction: move extra
    waits onto NoOps injected immediately before, on the same engine."""
    import concourse.mybir as mybir
    nsplit = 0
    for f in nc.m.functions:
        for bb in f.blocks:
            il = bb.instructions
            i = 0
            while i < len(il):
                ins = il[i]
                si = ins.sync_info
                if si is not None and si.on_wait is not None and len(si.on_wait) > 1:
                    waits = list(si.on_wait)
                    for k, w in enumerate(waits[:-1]):
                        nop = mybir.InstNoOp(name=f"{ins.name}-ws{k}", ins=[], outs=[])
                        nop.engine = ins.engine
                        nop.sync_info = mybir.SyncInfo(on_wait=[w], on_update=[])
                        il.insert(i, nop)
                        i += 1
                        nsplit += 1
                    ins.sync_info = mybir.SyncInfo(
                        on_wait=[waits[-1]], on_update=list(si.on_update or []))
                i += 1
    return nsplit


def build_nc(loop=1):
    import concourse.bass as bass
    import concourse.mybir as mybir
    import concourse.tile as tile
    from contextlib import ExitStack
    from concourse.masks import make_identity

    _install_tile_patch()
    f32, bf16 = mybir.dt.float32, mybir.dt.bfloat16
    AF = mybir.ActivationFunctionType
    OP = mybir.AluOpType

    nc = bass.Bass()
    x_d = nc.dram_tensor("x", [S, D], bf16, kind="ExternalInput")
    wq_d = nc.dram_tensor("wq", [ND, P, 2 * P], bf16, kind="ExternalInput")
    wk_d = nc.dram_tensor("wk", [ND, P, HD], bf16, kind="ExternalInput")
    wv_d = nc.dram_tensor("wv", [ND, P, HD], bf16, kind="ExternalInput")
    wo_d = nc.dram_tensor("wo", [2, P, D], bf16, kind="ExternalInput")
    c4_d = nc.dram_tensor("c4", [P, S], f32, kind="ExternalInput")
    s4_d = nc.dram_tensor("s4", [P, S], f32, kind="ExternalInput")
    cs2_d = nc.dram_tensor("cs2", [HD, S], f32, kind="ExternalInput")
    sc2_d = nc.dram_tensor("sc2", [HD, S], f32, kind="ExternalInput")
    tri_d = nc.dram_tensor("tri", [P, P], bf16, kind="ExternalInput")
    o_d = nc.dram_tensor("o", [S, D], bf16, kind="ExternalOutput")
    xs_d = (nc.dram_tensor("xs", [S, D], bf16, kind="Internal")
            if loop > 1 else None)

    with tile.TileContext(nc) as tc, ExitStack() as ctx:
        singles = ctx.enter_context(tc.tile_pool(name="singles", bufs=1))
        xpool = ctx.enter_context(tc.tile_pool(name="xpool", bufs=3))
        stats = ctx.enter_context(tc.tile_pool(name="stats", bufs=4))
        ropet = ctx.enter_context(tc.tile_pool(name="ropet", bufs=3))
        exppool = ctx.enter_context(tc.tile_pool(name="exppool", bufs=2))
        opool = ctx.enter_context(tc.tile_pool(name="opool", bufs=3))
        ps_proj = ctx.enter_context(tc.tile_pool(name="ps_proj", bufs=2, space="PSUM"))
        ps_sc = ctx.enter_context(tc.tile_pool(name="ps_sc", bufs=2, space="PSUM"))
        ps_pv = ctx.enter_context(tc.tile_pool(name="ps_pv", bufs=1, space="PSUM"))
        ps_tp = ctx.enter_context(tc.tile_pool(name="ps_tp", bufs=1, space="PSUM"))

        # ---- persistent SBUF constants ----
        wq_sb = singles.tile([P, ND, 2 * P], bf16)
        nc.sync.dma_start(out=wq_sb, in_=wq_d[:].rearrange("a p c -> p a c"))
        wk_sb = singles.tile([P, ND, HD], bf16)
        nc.sync.dma_start(out=wk_sb, in_=wk_d[:].rearrange("a p c -> p a c"))
        wv_sb = singles.tile([P, ND, HD], bf16)
        nc.sync.dma_start(out=wv_sb, in_=wv_d[:].rearrange("a p c -> p a c"))
        wo_sb = singles.tile([P, 2, D], bf16)
        nc.sync.dma_start(out=wo_sb, in_=wo_d[:].rearrange("a p c -> p a c"))
        c4_sb = singles.tile([P, S], f32)
        nc.sync.dma_start(out=c4_sb, in_=c4_d[:])
        s4_sb = singles.tile([P, S], f32)
        nc.sync.dma_start(out=s4_sb, in_=s4_d[:])
        cs2_sb = singles.tile([HD, S], f32)
        nc.sync.dma_start(out=cs2_sb, in_=cs2_d[:])
        sc2_sb = singles.tile([HD, S], f32)
        nc.sync.dma_start(out=sc2_sb, in_=sc2_d[:])
        tri_sb = singles.tile([P, P], bf16)
        nc.sync.dma_start(out=tri_sb, in_=tri_d[:])
        ident = singles.tile([P, P], bf16)
        make_identity(nc, ident)
        eps_sb = singles.tile([P, 1], f32)
        nc.vector.memset(eps_sb, EPS)

        # ---- persistent SBUF intermediates ----
        xnT = singles.tile([P, ND, NT, P], bf16)          # transposed normed x
        qre = singles.tile([P, S], bf16)                  # rotated q, re-half all heads
        qim = singles.tile([P, S], bf16)
        qhead = [singles.tile([P, S], bf16, name=f"qh{h}") for h in range(HG)]
        khead = singles.tile([P, S], bf16)
        v_sb = singles.tile([P, NT, HD + 1], bf16)
        nc.vector.memset(v_sb, 0.0)
        ctx_pair = [singles.tile([P, NT, P], bf16, name=f"ctxp{p}") for p in range(2)]
        ctxT = [singles.tile([P, NT, P], bf16, name=f"ctxT{p}") for p in range(2)]
        nc.vector.memset(v_sb[:, :, HD:HD + 1], 1.0)



        def body(src_d, dst_d):
            def ln_stats_all():
                # pass 1: stats for every tile; a single Sqrt per iteration
                # keeps the ACT table set parked on Exp for the whole
                # attention phase (a Sqrt<->Exp set switch costs ~2.7us)
                mvall = stats.tile([P, NT, 2], f32, tag="mvall")
                rstdall = stats.tile([P, NT, 1], f32, tag="rstdall")
                for tt in range(NT):
                    xt = xpool.tile([P, D], bf16, tag="xt", bufs=5)
                    nc.sync.dma_start(out=xt, in_=src_d[tt * P:(tt + 1) * P, :])
                    st = stats.tile([P, 2, 6], f32, tag="st")
                    xr = xt.rearrange("p (a b) -> p a b", a=2)
                    for a in range(2):
                        nc.vector.bn_stats(out=st[:, a, :], in_=xr[:, a, :])
                    nc.vector.bn_aggr(out=mvall[:, tt, :], in_=st)
                    if tt == 3:
                        nc.scalar.activation(out=rstdall[:, 0:4, :],
                                             in_=mvall[:, 0:4, 1:2],
                                             func=AF.Sqrt, bias=eps_sb,
                                             scale=1.0, alpha=0.0)
                        nc.vector.reciprocal(out=rstdall[:, 0:4, :],
                                             in_=rstdall[:, 0:4, :])
                nc.scalar.activation(out=rstdall[:, 4:NT, :],
                                     in_=mvall[:, 4:NT, 1:2],
                                     func=AF.Sqrt, bias=eps_sb, scale=1.0,
                                     alpha=0.0)
                nc.vector.reciprocal(out=rstdall[:, 4:NT, :],
                                     in_=rstdall[:, 4:NT, :])
                return mvall, rstdall

            def ln_chunk(c, mvall, rstdall):
                for tt in range(4 * c, 4 * (c + 1)):
                    xt = xpool.tile([P, D], bf16, tag="xt", bufs=5)
                    nc.sync.dma_start(out=xt, in_=src_d[tt * P:(tt + 1) * P, :])
                    xn = xpool.tile([P, D], bf16, tag="xn")
                    nc.vector.tensor_scalar(out=xn, in0=xt,
                                            scalar1=mvall[:, tt, 0:1],
                                            scalar2=rstdall[:, tt, :],
                                            op0=OP.subtract, op1=OP.mult)
                    nc.sync.dma_start_transpose(xnT[:, :, tt, :], xn)

            def v_tile(tt):
                pv = ps_proj.tile([P, CH], f32, tag="ps")
                for dblk in range(ND):
                    nc.tensor.matmul(pv[:, 0:HD], lhsT=xnT[:, dblk, tt, :],
                                     rhs=wv_sb[:, dblk, :],
                                     start=(dblk == 0), stop=(dblk == ND - 1))
                nc.vector.tensor_copy(v_sb[:, tt, 0:HD], pv[:, 0:HD])

            def qk_chunk(c):
                sl = slice(c * CH, (c + 1) * CH)
                pre = ps_proj.tile([P, CH], f32, tag="ps")
                pim = ps_proj.tile([P, CH], f32, tag="ps")
                pk = ps_proj.tile([P, CH], f32, tag="ps")
                for dblk in range(ND):
                    nc.tensor.matmul(pre, lhsT=wq_sb[:, dblk, 0:P],
                                     rhs=xnT[:, dblk, 4 * c:4 * (c + 1), :],
                                     start=(dblk == 0), stop=(dblk == ND - 1))
                for dblk in range(ND):
                    nc.tensor.matmul(pim, lhsT=wq_sb[:, dblk, P:2 * P],
                                     rhs=xnT[:, dblk, 4 * c:4 * (c + 1), :],
                                     start=(dblk == 0), stop=(dblk == ND - 1))
                for dblk in range(ND):
                    nc.tensor.matmul(pk[0:HD, :], lhsT=wk_sb[:, dblk, :],
                                     rhs=xnT[:, dblk, 4 * c:4 * (c + 1), :],
                                     start=(dblk == 0), stop=(dblk == ND - 1))
                # q rope: re' = re*c - im*s ; im' = re*s + im*c
                t1 = ropet.tile([P, CH], bf16, tag="t1")
                t2 = ropet.tile([P, CH], bf16, tag="t2")
                t3 = ropet.tile([P, CH], bf16, tag="t3")
                t4 = ropet.tile([P, CH], bf16, tag="t4")
                nc.vector.tensor_tensor(t1, pre, c4_sb[:, sl], OP.mult)
                nc.vector.tensor_tensor(t2, pim, s4_sb[:, sl], OP.mult)
                nc.vector.tensor_tensor(t3, pre, s4_sb[:, sl], OP.mult)
                nc.vector.tensor_tensor(t4, pim, c4_sb[:, sl], OP.mult)
                nc.vector.tensor_tensor(qre[:, sl], t1, t2, OP.subtract)
                nc.vector.tensor_tensor(qim[:, sl], t3, t4, OP.add)
                # k rope: stage re/im halves at base partition 0 (DVE is
                # lane-aligned; cross-partition moves go through DMA)
                ks = ropet.tile([HD, CH], bf16, tag="ks")
                nc.vector.tensor_copy(ks, pk[0:HD, :])
                ksi = ropet.tile([32, CH], bf16, tag="ksi")
                nc.sync.dma_start(out=ksi, in_=ks[32:HD, :])
                ta = ropet.tile([32, CH], bf16, tag="ta")
                tb = ropet.tile([32, CH], bf16, tag="tb")
                nc.vector.tensor_tensor(ta, ks[0:32, :], cs2_sb[0:32, sl], OP.mult)
                nc.vector.tensor_tensor(tb, ksi, sc2_sb[0:32, sl], OP.mult)
                nc.vector.tensor_tensor(khead[0:32, sl], ta, tb, OP.subtract)
                nc.vector.tensor_tensor(ta, ks[0:32, :], sc2_sb[0:32, sl], OP.mult)
                nc.vector.tensor_tensor(tb, ksi, cs2_sb[0:32, sl], OP.mult)
                kim = ropet.tile([32, CH], bf16, tag="kim")
                nc.vector.tensor_tensor(kim, ta, tb, OP.add)
                nc.sync.dma_start(out=khead[32:HD, sl], in_=kim)
                nc.sync.dma_start(out=khead[HD:2 * HD, sl],
                                  in_=khead[0:HD, sl])
                # per-chunk reshuffle of packed q into per-head tiles
                for h in range(HG):
                    nc.sync.dma_start(out=qhead[h][0:32, sl],
                                      in_=qre[32 * h:32 * (h + 1), sl])
                    nc.sync.dma_start(out=qhead[h][32:HD, sl],
                                      in_=qim[32 * h:32 * (h + 1), sl])
                    nc.sync.dma_start(out=qhead[h][HD:2 * HD, sl],
                                      in_=qhead[h][0:HD, sl])

            def attn_chunk(c):
                for h in range(HG):
                    expT = exppool.tile([P, NT, CH], bf16, tag="expT")
                    nblk = 4 * c + 4
                    for a in range(0, nblk, 2):   # key-block pairs
                        psc = ps_sc.tile([P, 2 * CH], f32, tag="psc")
                        for jj in range(2):
                            j = a + jj
                            off = max(0, P * (j - 4 * c))
                            lo, hi = jj * HD, (jj + 1) * HD
                            nc.tensor.matmul(
                                psc[:, jj * CH + off:(jj + 1) * CH],
                                lhsT=khead[lo:hi, j * P:(j + 1) * P],
                                rhs=qhead[h][lo:hi,
                                             c * CH + off:(c + 1) * CH],
                                start=True, stop=True)
                        nc.scalar.activation(out=expT[:, a:a + 2, :], in_=psc,
                                             func=AF.Exp, scale=0.125)
                    for j in range(4 * c, nblk):   # mask diagonal blocks
                        il = j - 4 * c
                        nc.gpsimd.tensor_tensor(
                            expT[:, j, il * P:(il + 1) * P],
                            expT[:, j, il * P:(il + 1) * P], tri_sb, OP.mult)
                    ppv = ps_pv.tile([P, 4, HD + 1], f32, tag="ppv")
                    for il in range(4):
                        iabs = 4 * c + il
                        for j in range(iabs + 1):
                            nc.tensor.matmul(ppv[:, il, :],
                                             lhsT=expT[:, j, il * P:(il + 1) * P],
                                             rhs=v_sb[:, j, :],
                                             start=(j == 0), stop=(j == iabs))
                    rec = stats.tile([P, 4, 1], f32, tag="rec")
                    nc.vector.reciprocal(out=rec, in_=ppv[:, :, HD:HD + 1])
                    pair, col = h // 2, (h % 2) * HD
                    nc.vector.tensor_tensor(
                        ctx_pair[pair][:, 4 * c:4 * (c + 1), col:col + HD],
                        ppv[:, :, 0:HD], rec.to_broadcast([P, 4, HD]), OP.mult)
                    if h == 1:
                        tp_pair(0, c)

            def tp_pair(pair, c):
                for tt in range(4 * c, 4 * (c + 1)):
                    ptp = ps_tp.tile([P, P], bf16, tag="ptp")
                    nc.tensor.transpose(ptp, ctx_pair[pair][:, tt, :], ident)
                    nc.vector.tensor_copy(ctxT[pair][:, tt, :], ptp)

            def outproj_chunk(c):
                tp_pair(1, c)
                for tt in range(4 * c, 4 * (c + 1)):
                    for half in range(2):
                        po = ps_proj.tile([P, CH], f32, tag="ps")
                        for pair in range(2):
                            nc.tensor.matmul(
                                po, lhsT=ctxT[pair][:, tt, :],
                                rhs=wo_sb[:, pair, half * CH:(half + 1) * CH],
                                start=(pair == 0), stop=(pair == 1))
                        ot = opool.tile([P, CH], bf16, tag="ot")
                        nc.vector.tensor_copy(ot, po)
                        nc.sync.dma_start(
                            out=dst_d[tt * P:(tt + 1) * P,
                                      half * CH:(half + 1) * CH],
                            in_=ot)

            # fused per-chunk pipeline; output projection runs one chunk
            # behind so it overlaps the next chunk's ACT-bound attention
            mvall, rstdall = ln_stats_all()
            for c in range(NCH):
                ln_chunk(c, mvall, rstdall)
                for tt in range(4 * c, 4 * (c + 1)):
                    v_tile(tt)
                qk_chunk(c)
                if c > 0:
                    outproj_chunk(c - 1)
                attn_chunk(c)
            outproj_chunk(NCH - 1)

        if loop == 1:
            body(x_d, o_d)
        else:
            nc.sync.dma_start(out=xs_d[:], in_=x_d[:])
            with tc.For_i(0, loop, 1):
                body(xs_d, xs_d)
            nc.sync.dma_start(out=o_d[:], in_=xs_d[:])
    n = _split_multi_waits(nc)
    print(f"kernel build(loop={loop}): split {n} extra sync-waits onto nops")
    return nc


def _prep_inputs(x, wq, wk, wv, wo, ln_w, ln_b, freqs_cos, freqs_sin):
    import ml_dtypes
    bf16 = ml_dtypes.bfloat16
    lnw = np.asarray(ln_w, np.float32)
    lnb = np.asarray(ln_b, np.float32)
    assert not np.any(lnb), "ln_b folding not implemented for nonzero bias"
    wq_f = lnw[:, None] * np.asarray(wq, np.float32)
    wk_f = lnw[:, None] * np.asarray(wk, np.float32)
    wv_f = lnw[:, None] * np.asarray(wv, np.float32)
    wo_f = np.asarray(wo, np.float32)
    cosT = np.ascontiguousarray(np.asarray(freqs_cos, np.float32).T)  # [32,S]
    sinT = np.ascontiguousarray(np.asarray(freqs_sin, np.float32).T)
    c4 = np.tile(cosT, (4, 1))
    s4 = np.tile(sinT, (4, 1))
    cs2 = np.vstack([cosT, sinT])
    sc2 = np.vstack([sinT, cosT])
    tri = (np.arange(P)[None, :] >= np.arange(P)[:, None]).astype(bf16)
    evens = [2 * i for i in range(32)]
    odds = [2 * i + 1 for i in range(32)]
    qperm = ([h * HD + e for h in range(HG) for e in evens]
             + [h * HD + o for h in range(HG) for o in odds])
    kperm = evens + odds
    in_maps = []
    for c in range(8):
        b, g = c // 4, c % 4
        wq_g = wq_f[:, g * 256:(g + 1) * 256][:, qperm]
        wk_g = wk_f[:, g * HD:(g + 1) * HD][:, kperm]
        wv_g = wv_f[:, g * HD:(g + 1) * HD]
        wo_g = wo_f[g * 256:(g + 1) * 256, :]
        in_maps.append({
            "x": np.ascontiguousarray(np.asarray(x, np.float32)[b].astype(bf16)),
            "wq": np.ascontiguousarray(wq_g.reshape(ND, P, 2 * P).astype(bf16)),
            "wk": np.ascontiguousarray(wk_g.reshape(ND, P, HD).astype(bf16)),
            "wv": np.ascontiguousarray(wv_g.reshape(ND, P, HD).astype(bf16)),
            "wo": np.ascontiguousarray(wo_g.reshape(2, P, D).astype(bf16)),
            "c4": c4, "s4": s4, "cs2": cs2, "sc2": sc2,
            "tri": np.ascontiguousarray(tri),
        })
    return in_maps


class _Runner:
    """Build the Bass module once and keep one jitted shard_map executable;
    repeat calls only pay input transfer + execution."""

    def __init__(self):
        import jax
        import jax.numpy as jnp
        from jax.sharding import Mesh, PartitionSpec
        from jax.experimental.shard_map import shard_map
        import concourse.mybir as mybir
        from concourse import bass2jax

        bass2jax.install_neuronx_cc_hook()
        nc = build_nc()
        nc_loop = build_nc(loop=LOOP_K)
        self.nc = nc
        in_names, out_names, out_avals, zero_outs = [], [], [], []
        pname = nc.partition_id_tensor.name if nc.partition_id_tensor else None
        for alloc in nc.m.functions[0].allocations:
            if not isinstance(alloc, mybir.MemoryLocationSet):
                continue
            name = alloc.memorylocations[0].name
            if alloc.kind == "ExternalInput" and name != pname:
                in_names.append(name)
            elif alloc.kind == "ExternalOutput":
                out_names.append(name)
                shape = tuple(alloc.tensor_shape)
                dt = mybir.dt.np(alloc.dtype)
                out_avals.append(jax.core.ShapedArray(shape, dt))
                zero_outs.append(np.zeros(shape, dt))
        self.in_names, self.out_names = list(in_names), out_names
        n_params = len(in_names)
        all_in = in_names + out_names
        if pname is not None:
            all_in = all_in + [pname]

        def _make_body(module):
            def _body(*args):
                operands = list(args)
                if pname is not None:
                    operands.append(bass2jax.partition_id_tensor())
                return tuple(bass2jax._bass_exec_p.bind(
                    *operands, out_avals=tuple(out_avals), in_names=tuple(all_in),
                    out_names=tuple(out_names), lowering_input_output_aliases=(),
                    sim_require_finite=True, sim_require_nnan=True, nc=module))
            return _body

        devices = jax.devices()[:8]
        self.mesh = Mesh(np.asarray(devices), ("core",))
        nin = n_params + len(out_names)
        self.fn = jax.jit(shard_map(
            _make_body(nc), mesh=self.mesh,
            in_specs=(PartitionSpec("core"),) * nin,
            out_specs=(PartitionSpec("core"),) * len(out_names),
            check_rep=False), keep_unused=True)
        self.fn_loop = jax.jit(shard_map(
            _make_body(nc_loop), mesh=self.mesh,
            in_specs=(PartitionSpec("core"),) * nin,
            out_specs=(PartitionSpec("core"),) * len(out_names),
            check_rep=False), keep_unused=True)

        # Chained-execution bookkeeping: x and o are both [S, D] bf16, so a
        # later execution can consume an earlier one's output directly.
        self.n_chain_inv = 2
        self.n_chain = self.n_chain_inv * LOOP_K
        self.x_idx = in_names.index("x")
        self.o_idx = out_names.index("o")
        self.zero_outs = zero_outs
        self.out_avals = out_avals

    def concat_inputs(self, in_maps):
        """Stage the per-core inputs on the devices (sharded along axis 0).
        Steady-state weights/activations live device-side; run() only
        dispatches the executable."""
        import jax
        from jax.sharding import NamedSharding, PartitionSpec
        cat = [np.concatenate([np.asarray(m[n]) for m in in_maps], axis=0)
               for n in self.in_names]
        cat += [np.zeros((8 * z.shape[0], *z.shape[1:]), z.dtype)
                for z in self.zero_outs]
        sh = NamedSharding(self.mesh, PartitionSpec("core"))
        cat = [jax.device_put(a, sh) for a in cat]
        jax.block_until_ready(cat)
        return cat

    def run(self, cat):
        import jax
        outs = self.fn(*cat)
        return jax.block_until_ready(outs)

    def run_loop(self, cat):
        """One invocation of the loop variant = LOOP_K serial executions of
        the kernel body on device (iteration i+1 consumes iteration i's
        output in a DRAM scratch)."""
        import jax
        outs = self.fn_loop(*cat)
        return jax.block_until_ready(outs)

    def run_chain(self, cat):
        """n_chain_inv invocations of the loop variant, where invocation
        i+1 consumes invocation i's output as its x. In total
        n_chain_inv * LOOP_K real executions with one completion round-trip;
        wall / n_chain is the per-execution device time."""
        import jax
        ops = list(cat)
        outs = None
        for _ in range(self.n_chain_inv):
            outs = self.fn_loop(*ops)
            ops[self.x_idx] = outs[self.o_idx]
        return jax.block_until_ready(outs)

    def to_host(self, outs):
        return [
            {n: np.asarray(outs[i]).reshape(8, *self.out_avals[i].shape)[c]
             for i, n in enumerate(self.out_names)}
            for c in range(8)
        ]


def get_runner():
    if "runner" not in _CACHE:
        _CACHE["runner"] = _Runner()
    return _CACHE["runner"]


def kernel(x, wq, wk, wv, wo, ln_w, ln_b, freqs_cos, freqs_sin, start_pos=0):
    r = get_runner()
    in_maps = _prep_inputs(x, wq, wk, wv, wo, ln_w, ln_b, freqs_cos, freqs_sin)
    cat = r.concat_inputs(in_maps)
    try:
        results = r.to_host(r.run(cat))
    except Exception:
        # first execution after a failed compile sometimes reports
        # NRT_EXEC_UNIT_UNRECOVERABLE; one retry clears it
        import time as _t
        _t.sleep(2.0)
        results = r.to_host(r.run(cat))
    out = np.zeros((B, S, D), np.float32)
    for c in range(8):
        out[c // 4] += np.asarray(results[c]["o"], np.float32)
    return out
